# revision 3
# baseline (speedup 1.0000x reference)
"""CLAPP/CPC hinge contrastive loss on 8 Trainium2 NeuronCores.

Strategy (SPMD, no collectives, no DRAM intermediates):
  - Reformulate score = ctx_i . (W_k z_s) as (ctx_i W_k) . z_s. Each core
    projects only ITS destination-context rows through W_k on TensorE
    (cheap: ~1/64 of the naive all-rows projection) and gathers raw z rows
    directly from a host-transposed z table in DRAM -- the 40MB per-core
    flat = W_k z intermediate of the naive scheme disappears entirely.
  - The (y,x,b) destination space is sharded 8 ways. Per 128-dest tile the
    17 rows per dest (16 negatives + 1 positive) are dma_gathered in
    TRANSPOSED layout [C-split, pair-cols], multiplied against the
    projected ctxW columns with a single 2x-mode DVE tensor_tensor
    (broadcast over the 17 pair groups), and reduced across the C
    partitions with ones-vector matmuls on TensorE accumulating straight
    into PSUM [128 dests, 17 scores].
  - ScalarE applies the hinge from PSUM (relu(1+s) / relu(1-s)) into a
    per-k accumulator; one activation-accumulate per k produces the
    partial sums. Host sums the 8 x [128,10] partials (the "all-reduce").
"""

import numpy as np
import ml_dtypes

B, C, H, Wd = 16, 256, 32, 32
K, NEG, SKIP = 5, 16, 1
S = H * Wd                      # 1024 spatial positions
ROWS = S * B                    # 16384 rows in the z table
NCORES = 8
N_K = [(H - k - SKIP) * Wd * B for k in range(1, K + 1)]   # [15360 ... 13312]
NPC = [n // NCORES for n in N_K]                           # dest rows per core
TILES_K = [(n + 127) // 128 for n in NPC]                  # [15, 15, 14, 14, 13]
T_TOT = sum(TILES_K)                                       # 71
PAIRS_PER_TILE = 16 * 128 + 128                            # 2048 neg + 128 pos
IDX_COLS = PAIRS_PER_TILE // 16                            # 136 idx columns/tile

_CACHE = {}


def _build():
    import concourse.bacc as bacc
    import concourse.mybir as mybir
    import concourse.tile as tile
    from contextlib import ExitStack

    bf16 = mybir.dt.bfloat16
    f32 = mybir.dt.float32
    i16 = mybir.dt.int16
    Copy = mybir.ActivationFunctionType.Copy
    Relu = mybir.ActivationFunctionType.Relu

    nc = bacc.Bacc("TRN2", target_bir_lowering=False, debug=False, num_devices=NCORES)

    zt = nc.dram_tensor("zt", [ROWS, 256], bf16, kind="ExternalInput")
    wt = nc.dram_tensor("wt", [2, 128, K * 256], bf16, kind="ExternalInput")
    ctxT = nc.dram_tensor("ctxT", [2, 128, T_TOT * 128], bf16, kind="ExternalInput")
    gidx = nc.dram_tensor("gidx", [128, T_TOT * IDX_COLS], i16, kind="ExternalInput")
    out = nc.dram_tensor("out", [128, 2 * K], f32, kind="ExternalOutput")

    with tile.TileContext(nc) as tc, ExitStack() as ctx:
        sb = ctx.enter_context(tc.tile_pool(name="sb", bufs=1))
        gp = ctx.enter_context(tc.tile_pool(name="gp", bufs=3))
        pp = ctx.enter_context(tc.tile_pool(name="pp", bufs=3))
        cwp = ctx.enter_context(tc.tile_pool(name="cwp", bufs=3))
        mmw = ctx.enter_context(tc.tile_pool(name="mmw", bufs=4, space="PSUM"))
        mms = ctx.enter_context(tc.tile_pool(name="mms", bufs=4, space="PSUM"))

        # ---- resident tensors ----
        wt_sb = [sb.tile([128, K * 256], bf16, name=f"wt{h}") for h in range(2)]
        ctxT_sb = [sb.tile([128, T_TOT * 128], bf16, name=f"ctxT{h}") for h in range(2)]
        for h in range(2):
            nc.sync.dma_start(wt_sb[h][:], wt[h])
            nc.sync.dma_start(ctxT_sb[h][:], ctxT[h])
        gidx_sb = sb.tile([128, T_TOT * IDX_COLS], i16, name="gidx_sb")
        nc.sync.dma_start(gidx_sb[:], gidx[:, :])
        ones = sb.tile([128, 1], bf16, name="ones")
        nc.vector.memset(ones[:], 1.0)
        # hinge values; pads must stay zero (memset once)
        hbuf = sb.tile([128, T_TOT, 17], bf16, name="hbuf")
        nc.vector.memset(hbuf[:], 0.0)
        out_sb = sb.tile([128, 2 * K], f32, name="out_sb")

        tglob = 0
        for k in range(1, K + 1):
            tk = TILES_K[k - 1]
            npc = NPC[k - 1]
            for t in range(tk):
                nv = min(128, npc - t * 128)
                # ctxW^T[e, d] = sum_o W[k,o,e] * ctx[d,o] on TensorE
                psW = mmw.tile([128, 2, 128], f32, tag="psW")
                for ec in range(2):
                    o0 = (k - 1) * 256 + ec * 128
                    for h in range(2):
                        nc.tensor.matmul(
                            psW[:, ec, :],
                            wt_sb[h][:, o0:o0 + 128],
                            ctxT_sb[h][:, tglob * 128:(tglob + 1) * 128],
                            start=(h == 0), stop=(h == 1))
                cw = cwp.tile([128, 2, 128], bf16, tag="cw")
                nc.scalar.activation(cw[:], psW[:], Copy)
                # gather 17 z rows per dest, transposed: [C-split, pair cols]
                g = gp.tile([128, 2, PAIRS_PER_TILE], bf16, tag="g")
                nc.gpsimd.dma_gather(
                    g[:], zt[:, :],
                    gidx_sb[:, tglob * IDX_COLS:(tglob + 1) * IDX_COLS],
                    PAIRS_PER_TILE, PAIRS_PER_TILE, 256,
                    transpose=True, single_packet=False)
                # P[p, j, n, d] = g * ctxW^T (broadcast over n) -- 2x DVE mode
                P = pp.tile([128, 2, 17, 128], bf16, tag="P")
                gv = g[:].rearrange("p j (n d) -> p j n d", n=17)
                cwb = (cw[:].rearrange("p j (o d) -> p j o d", o=1)
                       .broadcast_to([128, 2, 17, 128]))
                nc.vector.tensor_tensor(P[:], gv, cwb, mybir.AluOpType.mult)
                # scores[d, n] = sum over C: ones-matmul partition reduce
                ps = mms.tile([128, 17], f32, tag="ps")
                for n in range(17):
                    for j in range(2):
                        nc.tensor.matmul(ps[:, n:n + 1], P[:, j, n, :], ones[:],
                                         start=(j == 0), stop=(j == 1))
                # hinge straight from PSUM; only valid dests written
                nc.scalar.activation(hbuf[:nv, tglob, 0:16], ps[:nv, 0:16],
                                     Relu, bias=1.0, scale=1.0)
                nc.scalar.activation(hbuf[:nv, tglob, 16:17], ps[:nv, 16:17],
                                     Relu, bias=1.0, scale=-1.0)
                tglob += 1

        # per-k accumulation into output partials
        junk = sb.tile([128, 17 * 15], f32, name="junk")
        tg0 = 0
        for k in range(1, K + 1):
            tk = TILES_K[k - 1]
            nc.scalar.activation(junk[:, :tk * 16], hbuf[:, tg0:tg0 + tk, 0:16],
                                 Copy, scale=1.0 / (NEG * N_K[k - 1]),
                                 accum_out=out_sb[:, 2 * (k - 1) + 1:2 * (k - 1) + 2])
            nc.scalar.activation(junk[:, :tk], hbuf[:, tg0:tg0 + tk, 16:17],
                                 Copy, scale=1.0 / N_K[k - 1],
                                 accum_out=out_sb[:, 2 * (k - 1):2 * (k - 1) + 1])
            tg0 += tk
        nc.sync.dma_start(out[:, :], out_sb[:])

    nc.compile()
    return nc


def _host_prep(z, c, W, rand_index):
    """Build per-core input maps (host = sharding + layout only)."""
    zT = np.ascontiguousarray(
        z.transpose(2, 3, 0, 1).reshape(ROWS, 256)).astype(ml_dtypes.bfloat16)
    cT = np.ascontiguousarray(
        c.transpose(2, 3, 0, 1).reshape(ROWS, 256)).astype(ml_dtypes.bfloat16)
    wth = (W.reshape(K, 2, 128, 256).transpose(1, 2, 0, 3)
           .reshape(2, 128, K * 256).astype(ml_dtypes.bfloat16))

    in_maps = []
    for q in range(NCORES):
        ctxTq = np.zeros((2, 128, T_TOT * 128), dtype=ml_dtypes.bfloat16)
        gidxq = np.zeros((128, T_TOT * IDX_COLS), dtype=np.int16)
        tglob = 0
        for k in range(1, K + 1):
            nk, npc, tk = N_K[k - 1], NPC[k - 1], TILES_K[k - 1]
            base = 512 * (k + SKIP)          # z-row offset for step k
            ridx = rand_index[k - 1, : nk * NEG].astype(np.int64) % nk
            ridx = ridx.reshape(nk, NEG)     # [i, n] source rows (pre-offset)
            for t in range(tk):
                i0 = q * npc + t * 128
                nv = max(0, min(128, npc - t * 128))
                iglob = i0 + np.arange(128)
                iglob_c = np.minimum(iglob, nk - 1)          # clamp pads
                # ctx rows transposed (pads stay zero): NO base offset
                ctxTq[:, :, tglob * 128: tglob * 128 + nv] = (
                    cT[iglob[:nv]].T.reshape(2, 128, nv))
                # pair p = n*128 + d for negs; 2048+d for pos
                src = np.empty(PAIRS_PER_TILE, dtype=np.int16)
                src[:2048] = (ridx[iglob_c, :].T.reshape(2048) + base
                              ).astype(np.int16)
                src[2048:] = (iglob_c + base).astype(np.int16)
                gidxq[:, tglob * IDX_COLS:(tglob + 1) * IDX_COLS] = np.tile(
                    src.reshape(IDX_COLS, 16).T, (8, 1))
                tglob += 1
        in_maps.append({"zt": zT, "wt": wth, "ctxT": ctxTq, "gidx": gidxq})
    return in_maps


def kernel(z, c, W, rand_index):
    from concourse.bass_utils import run_bass_kernel_spmd

    if "nc" not in _CACHE:
        _CACHE["nc"] = _build()
    nc = _CACHE["nc"]
    in_maps = _host_prep(
        np.asarray(z, dtype=np.float32),
        np.asarray(c, dtype=np.float32),
        np.asarray(W, dtype=np.float32),
        np.asarray(rand_index),
    )
    res = run_bass_kernel_spmd(nc, in_maps, core_ids=list(range(NCORES)))
    _CACHE["last_res"] = res
    total = 0.0
    for r in res.results:
        total += r["out"].astype(np.float64).sum()
    return np.float32(total)


# revision 7
# speedup vs baseline: 1.0246x; 1.0246x over previous
"""CLAPP/CPC hinge contrastive loss on 8 Trainium2 NeuronCores.

Strategy (SPMD, no collectives, no DRAM intermediates):
  - Reformulate score = ctx_i . (W_k z_s) as (ctx_i W_k) . z_s. Each core
    projects only ITS destination-context rows through W_k on TensorE
    (cheap: ~1/64 of the naive all-rows projection) and gathers raw z rows
    directly from a host-transposed z table in DRAM -- the 40MB per-core
    flat = W_k z intermediate of the naive scheme disappears entirely.
  - The (y,x,b) destination space is sharded 8 ways. Per 128-dest tile the
    17 rows per dest (16 negatives + 1 positive) are dma_gathered in
    TRANSPOSED layout [C-split, pair-cols], multiplied against the
    projected ctxW columns with a single 2x-mode DVE tensor_tensor
    (broadcast over the 17 pair groups), and reduced across the C
    partitions with ones-vector matmuls on TensorE accumulating straight
    into PSUM [128 dests, 17 scores].
  - ScalarE applies the hinge from PSUM (relu(1+s) / relu(1-s)) into a
    per-k accumulator; one activation-accumulate per k produces the
    partial sums. Host sums the 8 x [128,10] partials (the "all-reduce").
"""

import numpy as np
import ml_dtypes

B, C, H, Wd = 16, 256, 32, 32
K, NEG, SKIP = 5, 16, 1
S = H * Wd                      # 1024 spatial positions
ROWS = S * B                    # 16384 rows in the z table
NCORES = 8
N_K = [(H - k - SKIP) * Wd * B for k in range(1, K + 1)]   # [15360 ... 13312]
NPC = [n // NCORES for n in N_K]                           # dest rows per core
TILES_K = [(n + 127) // 128 for n in NPC]                  # [15, 15, 14, 14, 13]
T_TOT = sum(TILES_K)                                       # 71
PAIRS_PER_TILE = 16 * 128 + 128                            # 2048 neg + 128 pos
IDX_COLS = PAIRS_PER_TILE // 16                            # 136 idx columns/tile

_CACHE = {}


def _build():
    import concourse.bacc as bacc
    import concourse.mybir as mybir
    import concourse.tile as tile
    from contextlib import ExitStack

    bf16 = mybir.dt.bfloat16
    f32 = mybir.dt.float32
    i16 = mybir.dt.int16
    fp8 = mybir.dt.float8e4
    Copy = mybir.ActivationFunctionType.Copy
    Relu = mybir.ActivationFunctionType.Relu

    nc = bacc.Bacc("TRN2", target_bir_lowering=False, debug=False, num_devices=NCORES)

    zt = nc.dram_tensor("zt", [ROWS, 256], bf16, kind="ExternalInput")
    wt = nc.dram_tensor("wt", [2, 128, K * 256], fp8, kind="ExternalInput")
    ctxT = nc.dram_tensor("ctxT", [2, 128, T_TOT * 128], fp8, kind="ExternalInput")
    gidx = nc.dram_tensor("gidx", [128, T_TOT * IDX_COLS], i16, kind="ExternalInput")
    out = nc.dram_tensor("out", [128, 2 * K], f32, kind="ExternalOutput")

    with tile.TileContext(nc) as tc, ExitStack() as ctx:
        sb = ctx.enter_context(tc.tile_pool(name="sb", bufs=1))
        gp = ctx.enter_context(tc.tile_pool(name="gp", bufs=3))
        pp = ctx.enter_context(tc.tile_pool(name="pp", bufs=3))
        cwp = ctx.enter_context(tc.tile_pool(name="cwp", bufs=3))
        mmw = ctx.enter_context(tc.tile_pool(name="mmw", bufs=4, space="PSUM"))
        mms = ctx.enter_context(tc.tile_pool(name="mms", bufs=4, space="PSUM"))

        # ---- resident tensors ----
        # per-k chunked loads so tile 0's gather/matmul start early
        kb = [0]
        for tk in TILES_K:
            kb.append(kb[-1] + tk)
        gidx_sb = sb.tile([128, T_TOT * IDX_COLS], i16, name="gidx_sb")
        for kk in range(K):
            c0, c1 = kb[kk] * IDX_COLS, kb[kk + 1] * IDX_COLS
            nc.sync.dma_start(gidx_sb[:, c0:c1], gidx[:, c0:c1])
        wt_sb = [sb.tile([128, K * 256], fp8, name=f"wt{h}") for h in range(2)]
        ctxT_sb = [sb.tile([128, T_TOT * 128], fp8, name=f"ctxT{h}") for h in range(2)]
        for h in range(2):
            nc.sync.dma_start(wt_sb[h][:], wt[h])
            for kk in range(K):
                c0, c1 = kb[kk] * 128, kb[kk + 1] * 128
                nc.sync.dma_start(ctxT_sb[h][:, c0:c1], ctxT[h][:, c0:c1])
        ones = sb.tile([128, 1], bf16, name="ones")
        nc.vector.memset(ones[:], 1.0)
        # hinge values; pads must stay zero (memset once)
        hbuf = sb.tile([128, T_TOT, 17], bf16, name="hbuf")
        nc.vector.memset(hbuf[:], 0.0)
        out_sb = sb.tile([128, 2 * K], f32, name="out_sb")
        junk = sb.tile([128, 17 * 15], f32, name="junk")

        tglob = 0
        for k in range(1, K + 1):
            tk = TILES_K[k - 1]
            npc = NPC[k - 1]
            for t in range(tk):
                nv = min(128, npc - t * 128)
                # ctxW^T[e, d] = sum_o W[k,o,e] * ctx[d,o] on TensorE
                psW = mmw.tile([128, 2, 128], f32, tag="psW")
                for ec in range(2):
                    o0 = (k - 1) * 256 + ec * 128
                    for h in range(2):
                        nc.tensor.matmul(
                            psW[:, ec, :],
                            wt_sb[h][:, o0:o0 + 128],
                            ctxT_sb[h][:, tglob * 128:(tglob + 1) * 128],
                            start=(h == 0), stop=(h == 1))
                cw = cwp.tile([128, 2, 128], bf16, tag="cw")
                # psW carries 256*ctxW^T (host pre-scales W by 256 to keep
                # fp8 operands out of subnormal range); undo here
                nc.scalar.activation(cw[:], psW[:], Copy, scale=1.0 / 256.0)
                # gather 17 z rows per dest, transposed: [C-split, pair cols]
                g = gp.tile([128, 2, PAIRS_PER_TILE], bf16, tag="g")
                nc.gpsimd.dma_gather(
                    g[:], zt[:, :],
                    gidx_sb[:, tglob * IDX_COLS:(tglob + 1) * IDX_COLS],
                    PAIRS_PER_TILE, PAIRS_PER_TILE, 256,
                    transpose=True, single_packet=False)
                # P[p, j, n, d] = g * ctxW^T (broadcast over n) -- 2x DVE mode
                P = pp.tile([128, 2, 17, 128], bf16, tag="P")
                gv = g[:].rearrange("p j (n d) -> p j n d", n=17)
                cwb = (cw[:].rearrange("p j (o d) -> p j o d", o=1)
                       .broadcast_to([128, 2, 17, 128]))
                nc.vector.tensor_tensor(P[:], gv, cwb, mybir.AluOpType.mult)
                # scores[d, n] = sum over C: ones-matmul partition reduce
                ps = mms.tile([128, 17], f32, tag="ps")
                for n in range(17):
                    for j in range(2):
                        nc.tensor.matmul(ps[:, n:n + 1], P[:, j, n, :], ones[:],
                                         start=(j == 0), stop=(j == 1))
                # hinge straight from PSUM; only valid dests written
                nc.scalar.activation(hbuf[:nv, tglob, 0:16], ps[:nv, 0:16],
                                     Relu, bias=1.0, scale=1.0)
                nc.scalar.activation(hbuf[:nv, tglob, 16:17], ps[:nv, 16:17],
                                     Relu, bias=1.0, scale=-1.0)
                tglob += 1
            # k's accumulation into output partials (overlaps later gathers)
            tg0 = tglob - tk
            nc.scalar.activation(junk[:, :tk * 16], hbuf[:, tg0:tg0 + tk, 0:16],
                                 Copy, scale=1.0 / (NEG * N_K[k - 1]),
                                 accum_out=out_sb[:, 2 * (k - 1) + 1:2 * (k - 1) + 2])
            nc.scalar.activation(junk[:, :tk], hbuf[:, tg0:tg0 + tk, 16:17],
                                 Copy, scale=1.0 / N_K[k - 1],
                                 accum_out=out_sb[:, 2 * (k - 1):2 * (k - 1) + 1])
        nc.sync.dma_start(out[:, :], out_sb[:])

    nc.compile()
    return nc


def _host_prep(z, c, W, rand_index):
    """Build per-core input maps (host = sharding + layout only)."""
    zT = np.ascontiguousarray(
        z.transpose(2, 3, 0, 1).reshape(ROWS, 256)).astype(ml_dtypes.bfloat16)
    cT = np.ascontiguousarray(
        c.transpose(2, 3, 0, 1).reshape(ROWS, 256)).astype(ml_dtypes.float8_e4m3)
    wth = ((W * 256.0).reshape(K, 2, 128, 256).transpose(1, 2, 0, 3)
           .reshape(2, 128, K * 256).astype(ml_dtypes.float8_e4m3))

    in_maps = []
    for q in range(NCORES):
        ctxTq = np.zeros((2, 128, T_TOT * 128), dtype=ml_dtypes.float8_e4m3)
        gidxq = np.zeros((128, T_TOT * IDX_COLS), dtype=np.int16)
        tglob = 0
        for k in range(1, K + 1):
            nk, npc, tk = N_K[k - 1], NPC[k - 1], TILES_K[k - 1]
            base = 512 * (k + SKIP)          # z-row offset for step k
            ridx = rand_index[k - 1, : nk * NEG].astype(np.int64) % nk
            ridx = ridx.reshape(nk, NEG)     # [i, n] source rows (pre-offset)
            for t in range(tk):
                i0 = q * npc + t * 128
                nv = max(0, min(128, npc - t * 128))
                iglob = i0 + np.arange(128)
                iglob_c = np.minimum(iglob, nk - 1)          # clamp pads
                # ctx rows transposed (pads stay zero): NO base offset
                ctxTq[:, :, tglob * 128: tglob * 128 + nv] = (
                    cT[iglob[:nv]].T.reshape(2, 128, nv))
                # pair p = n*128 + d for negs; 2048+d for pos
                src = np.empty(PAIRS_PER_TILE, dtype=np.int16)
                src[:2048] = (ridx[iglob_c, :].T.reshape(2048) + base
                              ).astype(np.int16)
                src[2048:] = (iglob_c + base).astype(np.int16)
                gidxq[:, tglob * IDX_COLS:(tglob + 1) * IDX_COLS] = np.tile(
                    src.reshape(IDX_COLS, 16).T, (8, 1))
                tglob += 1
        in_maps.append({"zt": zT, "wt": wth, "ctxT": ctxTq, "gidx": gidxq})
    return in_maps


def kernel(z, c, W, rand_index):
    from concourse.bass_utils import run_bass_kernel_spmd

    if "nc" not in _CACHE:
        _CACHE["nc"] = _build()
    nc = _CACHE["nc"]
    in_maps = _host_prep(
        np.asarray(z, dtype=np.float32),
        np.asarray(c, dtype=np.float32),
        np.asarray(W, dtype=np.float32),
        np.asarray(rand_index),
    )
    res = run_bass_kernel_spmd(nc, in_maps, core_ids=list(range(NCORES)))
    _CACHE["last_res"] = res
    total = 0.0
    for r in res.results:
        total += r["out"].astype(np.float64).sum()
    return np.float32(total)


# revision 11
# speedup vs baseline: 1.0444x; 1.0193x over previous
"""CLAPP/CPC hinge contrastive loss on 8 Trainium2 NeuronCores.

Strategy (SPMD, no collectives, no DRAM intermediates):
  - Reformulate score = ctx_i . (W_k z_s) as (ctx_i W_k) . z_s. Each core
    projects only ITS destination-context rows through W_k on TensorE
    (cheap: ~1/64 of the naive all-rows projection) and gathers raw z rows
    directly from a host-transposed z table in DRAM -- the 40MB per-core
    flat = W_k z intermediate of the naive scheme disappears entirely.
  - The (y,x,b) destination space is sharded 8 ways. Per 128-dest tile the
    17 rows per dest (16 negatives + 1 positive) are dma_gathered in
    TRANSPOSED layout [C-split, pair-cols], multiplied against the
    projected ctxW columns with a single 2x-mode DVE tensor_tensor
    (broadcast over the 17 pair groups), and reduced across the C
    partitions with ones-vector matmuls on TensorE accumulating straight
    into PSUM [128 dests, 17 scores].
  - ScalarE applies the hinge from PSUM (relu(1+s) / relu(1-s)) into a
    per-k accumulator; one activation-accumulate per k produces the
    partial sums. Host sums the 8 x [128,10] partials (the "all-reduce").
"""

import numpy as np
import ml_dtypes

B, C, H, Wd = 16, 256, 32, 32
K, NEG, SKIP = 5, 16, 1
S = H * Wd                      # 1024 spatial positions
ROWS = S * B                    # 16384 rows in the z table
NCORES = 8
N_K = [(H - k - SKIP) * Wd * B for k in range(1, K + 1)]   # [15360 ... 13312]
NPC = [n // NCORES for n in N_K]                           # dest rows per core
TILES_K = [(n + 127) // 128 for n in NPC]                  # [15, 15, 14, 14, 13]
T_TOT = sum(TILES_K)                                       # 71
PAIRS_PER_TILE = 16 * 128 + 128                            # 2048 neg + 128 pos
IDX_COLS = PAIRS_PER_TILE // 16                            # 136 idx columns/tile

# per-tile valid dest count / pair count / idx-col offset (half tiles gather
# only 17*64 pairs)
NV_T, NP_T, IC_OFF = [], [], [0]
for _k in range(1, K + 1):
    for _t in range(TILES_K[_k - 1]):
        _nv = min(128, NPC[_k - 1] - _t * 128)
        NV_T.append(_nv)
        NP_T.append((17 * _nv + 127) // 128 * 128)   # gather multiple of 128
        IC_OFF.append(IC_OFF[-1] + NP_T[-1] // 16)
IDX_TOT = IC_OFF[-1]

_CACHE = {}


def _build():
    import concourse.bacc as bacc
    import concourse.mybir as mybir
    import concourse.tile as tile
    from contextlib import ExitStack

    bf16 = mybir.dt.bfloat16
    f32 = mybir.dt.float32
    i16 = mybir.dt.int16
    fp8 = mybir.dt.float8e4
    Copy = mybir.ActivationFunctionType.Copy
    Relu = mybir.ActivationFunctionType.Relu

    nc = bacc.Bacc("TRN2", target_bir_lowering=False, debug=False, num_devices=NCORES)

    zt = nc.dram_tensor("zt", [ROWS, 256], bf16, kind="ExternalInput")
    wt = nc.dram_tensor("wt", [2, 128, K * 256], fp8, kind="ExternalInput")
    ctxT = nc.dram_tensor("ctxT", [2, 128, T_TOT * 128], fp8, kind="ExternalInput")
    gidx = nc.dram_tensor("gidx", [128, IDX_TOT], i16, kind="ExternalInput")
    out = nc.dram_tensor("out", [128, 2 * K], f32, kind="ExternalOutput")

    with tile.TileContext(nc) as tc, ExitStack() as ctx:
        sb = ctx.enter_context(tc.tile_pool(name="sb", bufs=1))
        gp = ctx.enter_context(tc.tile_pool(name="gp", bufs=3))
        pp = ctx.enter_context(tc.tile_pool(name="pp", bufs=3))
        cwp = ctx.enter_context(tc.tile_pool(name="cwp", bufs=3))
        mmw = ctx.enter_context(tc.tile_pool(name="mmw", bufs=4, space="PSUM"))
        mms = ctx.enter_context(tc.tile_pool(name="mms", bufs=4, space="PSUM"))

        # ---- resident tensors ----
        # per-k chunked loads so tile 0's gather/matmul start early
        kb = [0]
        for tk in TILES_K:
            kb.append(kb[-1] + tk)
        gidx_sb = sb.tile([128, IDX_TOT], i16, name="gidx_sb")
        for kk in range(K):
            c0, c1 = IC_OFF[kb[kk]], IC_OFF[kb[kk + 1]]
            nc.sync.dma_start(gidx_sb[:, c0:c1], gidx[:, c0:c1])
        wt_sb = [sb.tile([128, K * 256], fp8, name=f"wt{h}") for h in range(2)]
        ctxT_sb = [sb.tile([128, T_TOT * 128], fp8, name=f"ctxT{h}") for h in range(2)]
        for h in range(2):
            nc.sync.dma_start(wt_sb[h][:], wt[h])
            for kk in range(K):
                c0, c1 = kb[kk] * 128, kb[kk + 1] * 128
                nc.sync.dma_start(ctxT_sb[h][:, c0:c1], ctxT[h][:, c0:c1])
        ones = sb.tile([128, 1], bf16, name="ones")
        nc.vector.memset(ones[:], 1.0)
        # hinge values; pads must stay zero (memset once)
        hbuf = sb.tile([128, T_TOT, 17], bf16, name="hbuf")
        nc.vector.memset(hbuf[:], 0.0)
        out_sb = sb.tile([128, 2 * K], f32, name="out_sb")
        junk = sb.tile([128, 17 * 15], f32, name="junk")

        tglob = 0
        for k in range(1, K + 1):
            tk = TILES_K[k - 1]
            npc = NPC[k - 1]
            for t in range(tk):
                nv = min(128, npc - t * 128)
                # ctxW^T[e, d] = sum_o W[k,o,e] * ctx[d,o] on TensorE
                psW = mmw.tile([128, 2, 128], f32, tag="psW")
                for ec in range(2):
                    o0 = (k - 1) * 256 + ec * 128
                    for h in range(2):
                        nc.tensor.matmul(
                            psW[:, ec, :],
                            wt_sb[h][:, o0:o0 + 128],
                            ctxT_sb[h][:, tglob * 128:(tglob + 1) * 128],
                            start=(h == 0), stop=(h == 1))
                cw = cwp.tile([128, 2, 128], bf16, tag="cw")
                # psW carries 256*ctxW^T (host pre-scales W by 256 to keep
                # fp8 operands out of subnormal range); undo here
                nc.scalar.activation(cw[:], psW[:], Copy, scale=1.0 / 256.0)
                # gather 17 z rows per dest, transposed: [C-split, pair cols]
                npairs = NP_T[tglob]
                g = gp.tile([128, 2, npairs], bf16, tag=f"g{nv}")
                nc.gpsimd.dma_gather(
                    g[:], zt[:, :],
                    gidx_sb[:, IC_OFF[tglob]:IC_OFF[tglob + 1]],
                    npairs, npairs, 256,
                    transpose=True, single_packet=False)
                # P[p, j, n, d] = g * ctxW^T (broadcast over n) -- 2x DVE mode
                P = pp.tile([128, 2, 17, nv], bf16, tag=f"P{nv}")
                gv = g[:, :, :17 * nv].rearrange("p j (n d) -> p j n d", n=17)
                cwb = (cw[:, :, :nv].rearrange("p j (o d) -> p j o d", o=1)
                       .broadcast_to([128, 2, 17, nv]))
                nc.vector.tensor_tensor(P[:], gv, cwb, mybir.AluOpType.mult)
                # scores[d, n] = sum over C: ones-matmul partition reduce
                ps = mms.tile([128, 17], f32, tag="ps")
                for n in range(17):
                    for j in range(2):
                        nc.tensor.matmul(ps[:nv, n:n + 1], P[:, j, n, :], ones[:],
                                         start=(j == 0), stop=(j == 1))
                # hinge straight from PSUM; only valid dests written
                nc.scalar.activation(hbuf[:nv, tglob, 0:16], ps[:nv, 0:16],
                                     Relu, bias=1.0, scale=1.0)
                nc.scalar.activation(hbuf[:nv, tglob, 16:17], ps[:nv, 16:17],
                                     Relu, bias=1.0, scale=-1.0)
                tglob += 1
            # k's accumulation into output partials (overlaps later gathers)
            tg0 = tglob - tk
            nc.scalar.activation(junk[:, :tk * 16], hbuf[:, tg0:tg0 + tk, 0:16],
                                 Copy, scale=1.0 / (NEG * N_K[k - 1]),
                                 accum_out=out_sb[:, 2 * (k - 1) + 1:2 * (k - 1) + 2])
            nc.scalar.activation(junk[:, :tk], hbuf[:, tg0:tg0 + tk, 16:17],
                                 Copy, scale=1.0 / N_K[k - 1],
                                 accum_out=out_sb[:, 2 * (k - 1):2 * (k - 1) + 1])
        nc.sync.dma_start(out[:, :], out_sb[:])

    nc.compile()
    return nc


def _host_prep(z, c, W, rand_index):
    """Build per-core input maps (host = sharding + layout only)."""
    zT = np.ascontiguousarray(
        z.transpose(2, 3, 0, 1).reshape(ROWS, 256)).astype(ml_dtypes.bfloat16)
    cT = np.ascontiguousarray(
        c.transpose(2, 3, 0, 1).reshape(ROWS, 256)).astype(ml_dtypes.float8_e4m3)
    wth = ((W * 256.0).reshape(K, 2, 128, 256).transpose(1, 2, 0, 3)
           .reshape(2, 128, K * 256).astype(ml_dtypes.float8_e4m3))

    in_maps = []
    for q in range(NCORES):
        ctxTq = np.zeros((2, 128, T_TOT * 128), dtype=ml_dtypes.float8_e4m3)
        gidxq = np.zeros((128, IDX_TOT), dtype=np.int16)
        tglob = 0
        for k in range(1, K + 1):
            nk, npc, tk = N_K[k - 1], NPC[k - 1], TILES_K[k - 1]
            base = 512 * (k + SKIP)          # z-row offset for step k
            ridx = rand_index[k - 1, : nk * NEG].astype(np.int64) % nk
            ridx = ridx.reshape(nk, NEG)     # [i, n] source rows (pre-offset)
            for t in range(tk):
                i0 = q * npc + t * 128
                nv = max(0, min(128, npc - t * 128))
                iglob = i0 + np.arange(128)
                # ctx rows transposed (pads stay zero): NO base offset
                ctxTq[:, :, tglob * 128: tglob * 128 + nv] = (
                    cT[iglob[:nv]].T.reshape(2, 128, nv))
                # pair p = n*nv + d for negs; 16*nv+d for pos
                npairs = NP_T[tglob]
                src = np.full(npairs, base, dtype=np.int16)   # pads: any valid row
                src[:16 * nv] = (ridx[iglob[:nv], :].T.reshape(16 * nv) + base
                                 ).astype(np.int16)
                src[16 * nv:17 * nv] = (iglob[:nv] + base).astype(np.int16)
                gidxq[:, IC_OFF[tglob]:IC_OFF[tglob + 1]] = np.tile(
                    src.reshape(npairs // 16, 16).T, (8, 1))
                tglob += 1
        in_maps.append({"zt": zT, "wt": wth, "ctxT": ctxTq, "gidx": gidxq})
    return in_maps


def kernel(z, c, W, rand_index):
    from concourse.bass_utils import run_bass_kernel_spmd

    if "nc" not in _CACHE:
        _CACHE["nc"] = _build()
    nc = _CACHE["nc"]
    in_maps = _host_prep(
        np.asarray(z, dtype=np.float32),
        np.asarray(c, dtype=np.float32),
        np.asarray(W, dtype=np.float32),
        np.asarray(rand_index),
    )
    res = run_bass_kernel_spmd(nc, in_maps, core_ids=list(range(NCORES)))
    _CACHE["last_res"] = res
    total = 0.0
    for r in res.results:
        total += r["out"].astype(np.float64).sum()
    return np.float32(total)


# revision 15
# speedup vs baseline: 1.0722x; 1.0266x over previous
"""CLAPP/CPC hinge contrastive loss on 8 Trainium2 NeuronCores.

Strategy (SPMD, no collectives, no DRAM intermediates):
  - Reformulate score = ctx_i . (W_k z_s) as (ctx_i W_k) . z_s. Each core
    projects only ITS destination-context rows through W_k on TensorE
    (cheap: ~1/64 of the naive all-rows projection) and gathers raw z rows
    directly from a host-transposed z table in DRAM -- the 40MB per-core
    flat = W_k z intermediate of the naive scheme disappears entirely.
  - The (y,x,b) destination space is sharded 8 ways. Per 128-dest tile the
    17 rows per dest (16 negatives + 1 positive) are dma_gathered in
    TRANSPOSED layout [C-split, pair-cols], multiplied against the
    projected ctxW columns with a single 2x-mode DVE tensor_tensor
    (broadcast over the 17 pair groups), and reduced across the C
    partitions with ones-vector matmuls on TensorE accumulating straight
    into PSUM [128 dests, 17 scores].
  - ScalarE applies the hinge from PSUM (relu(1+s) / relu(1-s)) into a
    per-k accumulator; one activation-accumulate per k produces the
    partial sums. Host sums the 8 x [128,10] partials (the "all-reduce").
"""

import numpy as np
import ml_dtypes

B, C, H, Wd = 16, 256, 32, 32
K, NEG, SKIP = 5, 16, 1
S = H * Wd                      # 1024 spatial positions
ROWS = S * B                    # 16384 rows in the z table
NCORES = 8
N_K = [(H - k - SKIP) * Wd * B for k in range(1, K + 1)]   # [15360 ... 13312]
NPC = [n // NCORES for n in N_K]                           # dest rows per core
TILES_K = [(n + 127) // 128 for n in NPC]                  # [15, 15, 14, 14, 13]
T_TOT = sum(TILES_K)                                       # 71
PAIRS_PER_TILE = 16 * 128 + 128                            # 2048 neg + 128 pos
IDX_COLS = PAIRS_PER_TILE // 16                            # 136 idx columns/tile

# per-tile valid dest count / pair count / idx-col offset (half tiles gather
# only 17*64 pairs)
NV_T, NP_T, IC_OFF = [], [], [0]
for _k in range(1, K + 1):
    for _t in range(TILES_K[_k - 1]):
        _nv = min(128, NPC[_k - 1] - _t * 128)
        NV_T.append(_nv)
        NP_T.append((17 * _nv + 127) // 128 * 128)   # gather multiple of 128
        IC_OFF.append(IC_OFF[-1] + NP_T[-1] // 16)
IDX_TOT = IC_OFF[-1]
NPC_MIN = NPC[-1]                                          # 1664
CTX_COLS = (NCORES - 1) * (NPC[0] - NPC_MIN) + NPC[0]       # union window (3712)

_CACHE = {}


def _build():
    import concourse.bacc as bacc
    import concourse.mybir as mybir
    import concourse.tile as tile
    from contextlib import ExitStack

    bf16 = mybir.dt.bfloat16
    f32 = mybir.dt.float32
    i16 = mybir.dt.int16
    fp8 = mybir.dt.float8e4
    Copy = mybir.ActivationFunctionType.Copy
    Relu = mybir.ActivationFunctionType.Relu

    nc = bacc.Bacc("TRN2", target_bir_lowering=False, debug=False, num_devices=NCORES)

    zt = nc.dram_tensor("zt", [ROWS, 256], bf16, kind="ExternalInput")
    wt = nc.dram_tensor("wt", [2, 128, K * 256], fp8, kind="ExternalInput")
    ctxT = nc.dram_tensor("ctxT", [2, 128, CTX_COLS], fp8, kind="ExternalInput")
    gidx = nc.dram_tensor("gidx", [128, IDX_TOT], i16, kind="ExternalInput")
    out = nc.dram_tensor("out", [128, 2 * K], f32, kind="ExternalOutput")

    with tile.TileContext(nc) as tc, ExitStack() as ctx:
        sb = ctx.enter_context(tc.tile_pool(name="sb", bufs=1))
        gp = ctx.enter_context(tc.tile_pool(name="gp", bufs=5))
        pp = ctx.enter_context(tc.tile_pool(name="pp", bufs=4))
        cwp = ctx.enter_context(tc.tile_pool(name="cwp", bufs=3))
        mmw = ctx.enter_context(tc.tile_pool(name="mmw", bufs=5, space="PSUM"))
        mms = ctx.enter_context(tc.tile_pool(name="mms", bufs=3, space="PSUM"))

        # ---- resident tensors ----
        # per-k chunked loads so tile 0's gather/matmul start early
        kb = [0]
        for tk in TILES_K:
            kb.append(kb[-1] + tk)
        gidx_sb = sb.tile([128, IDX_TOT], i16, name="gidx_sb")
        nc.sync.dma_start(gidx_sb[:, :IC_OFF[1]], gidx[:, :IC_OFF[1]])
        for kk in range(K):
            c0, c1 = max(IC_OFF[1], IC_OFF[kb[kk]]), IC_OFF[kb[kk + 1]]
            if c1 > c0:
                nc.sync.dma_start(gidx_sb[:, c0:c1], gidx[:, c0:c1])
        wt_sb = [sb.tile([128, K * 256], fp8, name=f"wt{h}") for h in range(2)]
        # per-core union window of transposed ctx rows; q-dependent source
        # offset handled host-side is impossible (one NEFF for all cores),
        # so the FULL per-core window is supplied as a per-core input and
        # loaded whole; tiles index it with a register offset q*(npc_k-1664).
        ctxT_sb = [sb.tile([128, CTX_COLS], fp8, name=f"ctxT{h}") for h in range(2)]
        for h in range(2):
            nc.sync.dma_start(wt_sb[h][:], wt[h])
            nc.sync.dma_start(ctxT_sb[h][:], ctxT[h])
        from concourse.bass import ds
        q_pe = nc.tensor.partition_id()
        ones = sb.tile([128, 1], bf16, name="ones")
        nc.vector.memset(ones[:], 1.0)
        # hinge values; pads must stay zero (memset once)
        hbuf = sb.tile([128, T_TOT, 17], bf16, name="hbuf")
        nc.vector.memset(hbuf[:], 0.0)
        out_sb = sb.tile([128, 2 * K], f32, name="out_sb")
        junk = sb.tile([128, 17 * 15], f32, name="junk")

        for k in [1, 2, 3, 5, 4]:
            tglob = kb[k - 1]
            tk = TILES_K[k - 1]
            npc = NPC[k - 1]
            for t in range(tk):
                nv = min(128, npc - t * 128)
                # ctxW^T[e, d] = sum_o W[k,o,e] * ctx[d,o] on TensorE
                psW = mmw.tile([128, 2, 128], f32, tag="psW")
                coff = q_pe * (NPC[k - 1] - NPC_MIN) + t * 128
                for ec in range(2):
                    o0 = (k - 1) * 256 + ec * 128
                    for h in range(2):
                        nc.tensor.matmul(
                            psW[:, ec, :],
                            wt_sb[h][:, o0:o0 + 128],
                            ctxT_sb[h][:, ds(coff, 128)],
                            start=(h == 0), stop=(h == 1))
                cw = cwp.tile([128, 2, 128], bf16, tag="cw")
                # psW carries 256*ctxW^T (host pre-scales W by 256 to keep
                # fp8 operands out of subnormal range); undo here
                nc.scalar.activation(cw[:], psW[:], Copy, scale=1.0 / 256.0)
                # gather 17 z rows per dest, transposed: [C-split, pair cols]
                npairs = NP_T[tglob]
                g = gp.tile([128, 2, npairs], bf16, tag=f"g{nv}")
                nc.gpsimd.dma_gather(
                    g[:], zt[:, :],
                    gidx_sb[:, IC_OFF[tglob]:IC_OFF[tglob + 1]],
                    npairs, npairs, 256,
                    transpose=True, single_packet=False)
                # P[p, j, n, d] = g * ctxW^T (broadcast over n) -- 2x DVE mode
                P = pp.tile([128, 2, 17, nv], bf16, tag=f"P{nv}")
                gv = g[:, :, :17 * nv].rearrange("p j (n d) -> p j n d", n=17)
                cwb = (cw[:, :, :nv].rearrange("p j (o d) -> p j o d", o=1)
                       .broadcast_to([128, 2, 17, nv]))
                nc.vector.tensor_tensor(P[:], gv, cwb, mybir.AluOpType.mult)
                # scores[d, n] = sum over C: ones-matmul partition reduce
                ps = mms.tile([128, 17], f32, tag="ps")
                for n in range(17):
                    for j in range(2):
                        nc.tensor.matmul(ps[:nv, n:n + 1], P[:, j, n, :], ones[:],
                                         start=(j == 0), stop=(j == 1))
                # hinge straight from PSUM; only valid dests written
                nc.scalar.activation(hbuf[:nv, tglob, 0:16], ps[:nv, 0:16],
                                     Relu, bias=1.0, scale=1.0)
                nc.scalar.activation(hbuf[:nv, tglob, 16:17], ps[:nv, 16:17],
                                     Relu, bias=1.0, scale=-1.0)
                tglob += 1
            # k's accumulation into output partials (overlaps later gathers)
            tg0 = tglob - tk
            nc.scalar.activation(junk[:, :tk * 16], hbuf[:, tg0:tg0 + tk, 0:16],
                                 Copy, scale=1.0 / (NEG * N_K[k - 1]),
                                 accum_out=out_sb[:, 2 * (k - 1) + 1:2 * (k - 1) + 2])
            nc.scalar.activation(junk[:, :tk], hbuf[:, tg0:tg0 + tk, 16:17],
                                 Copy, scale=1.0 / N_K[k - 1],
                                 accum_out=out_sb[:, 2 * (k - 1):2 * (k - 1) + 1])
        nc.sync.dma_start(out[:, :], out_sb[:])

    nc.compile()
    return nc


def _host_prep(z, c, W, rand_index):
    """Build per-core input maps (host = sharding + layout only)."""
    zT = np.ascontiguousarray(
        z.transpose(2, 3, 0, 1).reshape(ROWS, 256)).astype(ml_dtypes.bfloat16)
    cT = np.ascontiguousarray(
        c.transpose(2, 3, 0, 1).reshape(ROWS, 256)).astype(ml_dtypes.float8_e4m3)
    wth = ((W * 256.0).reshape(K, 2, 128, 256).transpose(1, 2, 0, 3)
           .reshape(2, 128, K * 256).astype(ml_dtypes.float8_e4m3))

    in_maps = []
    ctx_cols_in = CTX_COLS
    for q in range(NCORES):
        lo = q * NPC_MIN
        hi = min(ROWS, lo + ctx_cols_in)
        ctxTq = np.zeros((2, 128, ctx_cols_in), dtype=ml_dtypes.float8_e4m3)
        ctxTq[:, :, :hi - lo] = cT[lo:hi].T.reshape(2, 128, hi - lo)
        gidxq = np.zeros((128, IDX_TOT), dtype=np.int16)
        tglob = 0
        for k in range(1, K + 1):
            nk, npc, tk = N_K[k - 1], NPC[k - 1], TILES_K[k - 1]
            base = 512 * (k + SKIP)          # z-row offset for step k
            ridx = rand_index[k - 1, : nk * NEG].astype(np.int64) % nk
            ridx = ridx.reshape(nk, NEG)     # [i, n] source rows (pre-offset)
            for t in range(tk):
                i0 = q * npc + t * 128
                nv = max(0, min(128, npc - t * 128))
                iglob = i0 + np.arange(128)
                # pair p = n*nv + d for negs; 16*nv+d for pos
                npairs = NP_T[tglob]
                src = np.full(npairs, base, dtype=np.int16)   # pads: any valid row
                src[:16 * nv] = (ridx[iglob[:nv], :].T.reshape(16 * nv) + base
                                 ).astype(np.int16)
                src[16 * nv:17 * nv] = (iglob[:nv] + base).astype(np.int16)
                gidxq[:, IC_OFF[tglob]:IC_OFF[tglob + 1]] = np.tile(
                    src.reshape(npairs // 16, 16).T, (8, 1))
                tglob += 1
        in_maps.append({"zt": zT, "wt": wth, "ctxT": ctxTq, "gidx": gidxq})
    return in_maps


def kernel(z, c, W, rand_index):
    from concourse.bass_utils import run_bass_kernel_spmd

    if "nc" not in _CACHE:
        _CACHE["nc"] = _build()
    nc = _CACHE["nc"]
    in_maps = _host_prep(
        np.asarray(z, dtype=np.float32),
        np.asarray(c, dtype=np.float32),
        np.asarray(W, dtype=np.float32),
        np.asarray(rand_index),
    )
    res = run_bass_kernel_spmd(nc, in_maps, core_ids=list(range(NCORES)))
    _CACHE["last_res"] = res
    total = 0.0
    for r in res.results:
        total += r["out"].astype(np.float64).sum()
    return np.float32(total)


# revision 16
# speedup vs baseline: 1.0730x; 1.0008x over previous
"""CLAPP/CPC hinge contrastive loss on 8 Trainium2 NeuronCores.

Strategy (SPMD, no collectives, no DRAM intermediates):
  - Reformulate score = ctx_i . (W_k z_s) as (ctx_i W_k) . z_s. Each core
    projects only ITS destination-context rows through W_k on TensorE
    (cheap: ~1/64 of the naive all-rows projection) and gathers raw z rows
    directly from a host-transposed z table in DRAM -- the 40MB per-core
    flat = W_k z intermediate of the naive scheme disappears entirely.
  - The (y,x,b) destination space is sharded 8 ways. Per 128-dest tile the
    17 rows per dest (16 negatives + 1 positive) are dma_gathered in
    TRANSPOSED layout [C-split, pair-cols], multiplied against the
    projected ctxW columns with a single 2x-mode DVE tensor_tensor
    (broadcast over the 17 pair groups), and reduced across the C
    partitions with ones-vector matmuls on TensorE accumulating straight
    into PSUM [128 dests, 17 scores].
  - ScalarE applies the hinge from PSUM (relu(1+s) / relu(1-s)) into a
    per-k accumulator; one activation-accumulate per k produces the
    partial sums. Host sums the 8 x [128,10] partials (the "all-reduce").
"""

import numpy as np
import ml_dtypes

B, C, H, Wd = 16, 256, 32, 32
K, NEG, SKIP = 5, 16, 1
S = H * Wd                      # 1024 spatial positions
ROWS = S * B                    # 16384 rows in the z table
NCORES = 8
N_K = [(H - k - SKIP) * Wd * B for k in range(1, K + 1)]   # [15360 ... 13312]
NPC = [n // NCORES for n in N_K]                           # dest rows per core
TILES_K = [(n + 127) // 128 for n in NPC]                  # [15, 15, 14, 14, 13]
T_TOT = sum(TILES_K)                                       # 71
PAIRS_PER_TILE = 16 * 128 + 128                            # 2048 neg + 128 pos
IDX_COLS = PAIRS_PER_TILE // 16                            # 136 idx columns/tile

# per-tile valid dest count / pair count / idx-col offset (half tiles gather
# only 17*64 pairs)
NV_T, NP_T, IC_OFF = [], [], [0]
for _k in range(1, K + 1):
    for _t in range(TILES_K[_k - 1]):
        _nv = min(128, NPC[_k - 1] - _t * 128)
        NV_T.append(_nv)
        NP_T.append((17 * _nv + 127) // 128 * 128)   # gather multiple of 128
        IC_OFF.append(IC_OFF[-1] + NP_T[-1] // 16)
IDX_TOT = IC_OFF[-1]
NPC_MIN = NPC[-1]                                          # 1664
CTX_COLS = (NCORES - 1) * (NPC[0] - NPC_MIN) + NPC[0]       # union window (3712)

_CACHE = {}


def _build():
    import concourse.bacc as bacc
    import concourse.mybir as mybir
    import concourse.tile as tile
    from contextlib import ExitStack

    bf16 = mybir.dt.bfloat16
    f32 = mybir.dt.float32
    i16 = mybir.dt.int16
    fp8 = mybir.dt.float8e4
    Copy = mybir.ActivationFunctionType.Copy
    Relu = mybir.ActivationFunctionType.Relu

    nc = bacc.Bacc("TRN2", target_bir_lowering=False, debug=False, num_devices=NCORES)

    zt = nc.dram_tensor("zt", [ROWS, 256], bf16, kind="ExternalInput")
    wt = nc.dram_tensor("wt", [2, 128, K * 256], fp8, kind="ExternalInput")
    ctxT = nc.dram_tensor("ctxT", [2, 128, CTX_COLS], fp8, kind="ExternalInput")
    gidx = nc.dram_tensor("gidx", [128, IDX_TOT], i16, kind="ExternalInput")
    out = nc.dram_tensor("out", [128, 2 * K + 2], f32, kind="ExternalOutput")

    with tile.TileContext(nc) as tc, ExitStack() as ctx:
        sb = ctx.enter_context(tc.tile_pool(name="sb", bufs=1))
        gp = ctx.enter_context(tc.tile_pool(name="gp", bufs=5))
        pp = ctx.enter_context(tc.tile_pool(name="pp", bufs=4))
        cwp = ctx.enter_context(tc.tile_pool(name="cwp", bufs=3))
        mmw = ctx.enter_context(tc.tile_pool(name="mmw", bufs=5, space="PSUM"))
        mms = ctx.enter_context(tc.tile_pool(name="mms", bufs=3, space="PSUM"))

        # ---- resident tensors ----
        # per-k chunked loads so tile 0's gather/matmul start early
        kb = [0]
        for tk in TILES_K:
            kb.append(kb[-1] + tk)
        gidx_sb = sb.tile([128, IDX_TOT], i16, name="gidx_sb")
        nc.sync.dma_start(gidx_sb[:, :IC_OFF[1]], gidx[:, :IC_OFF[1]])
        for kk in range(K):
            c0, c1 = max(IC_OFF[1], IC_OFF[kb[kk]]), IC_OFF[kb[kk + 1]]
            if c1 > c0:
                nc.sync.dma_start(gidx_sb[:, c0:c1], gidx[:, c0:c1])
        wt_sb = [sb.tile([128, K * 256], fp8, name=f"wt{h}") for h in range(2)]
        # per-core union window of transposed ctx rows; q-dependent source
        # offset handled host-side is impossible (one NEFF for all cores),
        # so the FULL per-core window is supplied as a per-core input and
        # loaded whole; tiles index it with a register offset q*(npc_k-1664).
        ctxT_sb = [sb.tile([128, CTX_COLS], fp8, name=f"ctxT{h}") for h in range(2)]
        for h in range(2):
            nc.sync.dma_start(wt_sb[h][:], wt[h])
            nc.sync.dma_start(ctxT_sb[h][:], ctxT[h])
        from concourse.bass import ds
        q_pe = nc.tensor.partition_id()
        ones = sb.tile([128, 1], bf16, name="ones")
        nc.vector.memset(ones[:], 1.0)
        # hinge values; pads must stay zero (memset once)
        hbuf = sb.tile([128, T_TOT, 17], bf16, name="hbuf")
        nc.vector.memset(hbuf[:], 0.0)
        out_sb = sb.tile([128, 2 * K + 2], f32, name="out_sb")
        junk = sb.tile([128, 17 * 15], f32, name="junk")

        for k in [1, 2, 3, 5, 4]:
            tglob = kb[k - 1]
            tk = TILES_K[k - 1]
            npc = NPC[k - 1]
            for t in range(tk):
                nv = min(128, npc - t * 128)
                # ctxW^T[e, d] = sum_o W[k,o,e] * ctx[d,o] on TensorE
                psW = mmw.tile([128, 2, 128], f32, tag="psW")
                coff = q_pe * (NPC[k - 1] - NPC_MIN) + t * 128
                for ec in range(2):
                    o0 = (k - 1) * 256 + ec * 128
                    for h in range(2):
                        nc.tensor.matmul(
                            psW[:, ec, :],
                            wt_sb[h][:, o0:o0 + 128],
                            ctxT_sb[h][:, ds(coff, 128)],
                            start=(h == 0), stop=(h == 1))
                cw = cwp.tile([128, 2, 128], bf16, tag="cw")
                # psW carries 256*ctxW^T (host pre-scales W by 256 to keep
                # fp8 operands out of subnormal range); undo here
                nc.scalar.activation(cw[:], psW[:], Copy, scale=1.0 / 256.0)
                # gather 17 z rows per dest, transposed: [C-split, pair cols]
                npairs = NP_T[tglob]
                g = gp.tile([128, 2, npairs], bf16, tag=f"g{nv}")
                nc.gpsimd.dma_gather(
                    g[:], zt[:, :],
                    gidx_sb[:, IC_OFF[tglob]:IC_OFF[tglob + 1]],
                    npairs, npairs, 256,
                    transpose=True, single_packet=False)
                # P[p, j, n, d] = g * ctxW^T (broadcast over n) -- 2x DVE mode
                P = pp.tile([128, 2, 17, nv], bf16, tag=f"P{nv}")
                gv = g[:, :, :17 * nv].rearrange("p j (n d) -> p j n d", n=17)
                cwb = (cw[:, :, :nv].rearrange("p j (o d) -> p j o d", o=1)
                       .broadcast_to([128, 2, 17, nv]))
                nc.vector.tensor_tensor(P[:], gv, cwb, mybir.AluOpType.mult)
                # scores[d, n] = sum over C: ones-matmul partition reduce
                ps = mms.tile([128, 17], f32, tag="ps")
                for n in range(17):
                    for j in range(2):
                        nc.tensor.matmul(ps[:nv, n:n + 1], P[:, j, n, :], ones[:],
                                         start=(j == 0), stop=(j == 1))
                # hinge straight from PSUM; only valid dests written
                nc.scalar.activation(hbuf[:nv, tglob, 0:16], ps[:nv, 0:16],
                                     Relu, bias=1.0, scale=1.0)
                nc.scalar.activation(hbuf[:nv, tglob, 16:17], ps[:nv, 16:17],
                                     Relu, bias=1.0, scale=-1.0)
                tglob += 1
            # k's accumulation into output partials (overlaps later gathers);
            # for the globally-last k (4) leave only the final tile's sliver
            # on the drain path, accumulated into the 2 extra out columns
            tg0 = tglob - tk
            ta = tk - 1 if k == 4 else tk
            nc.scalar.activation(junk[:, :ta * 16], hbuf[:, tg0:tg0 + ta, 0:16],
                                 Copy, scale=1.0 / (NEG * N_K[k - 1]),
                                 accum_out=out_sb[:, 2 * (k - 1) + 1:2 * (k - 1) + 2])
            nc.scalar.activation(junk[:, :ta], hbuf[:, tg0:tg0 + ta, 16:17],
                                 Copy, scale=1.0 / N_K[k - 1],
                                 accum_out=out_sb[:, 2 * (k - 1):2 * (k - 1) + 1])
            if k == 4:
                nc.scalar.activation(junk[:, :16], hbuf[:, tglob - 1, 0:16],
                                     Copy, scale=1.0 / (NEG * N_K[k - 1]),
                                     accum_out=out_sb[:, 2 * K + 1:2 * K + 2])
                nc.scalar.activation(junk[:, :1], hbuf[:, tglob - 1, 16:17],
                                     Copy, scale=1.0 / N_K[k - 1],
                                     accum_out=out_sb[:, 2 * K:2 * K + 1])
        nc.sync.dma_start(out[:, :], out_sb[:])

    nc.compile()
    return nc


def _host_prep(z, c, W, rand_index):
    """Build per-core input maps (host = sharding + layout only)."""
    zT = np.ascontiguousarray(
        z.transpose(2, 3, 0, 1).reshape(ROWS, 256)).astype(ml_dtypes.bfloat16)
    cT = np.ascontiguousarray(
        c.transpose(2, 3, 0, 1).reshape(ROWS, 256)).astype(ml_dtypes.float8_e4m3)
    wth = ((W * 256.0).reshape(K, 2, 128, 256).transpose(1, 2, 0, 3)
           .reshape(2, 128, K * 256).astype(ml_dtypes.float8_e4m3))

    in_maps = []
    ctx_cols_in = CTX_COLS
    for q in range(NCORES):
        lo = q * NPC_MIN
        hi = min(ROWS, lo + ctx_cols_in)
        ctxTq = np.zeros((2, 128, ctx_cols_in), dtype=ml_dtypes.float8_e4m3)
        ctxTq[:, :, :hi - lo] = cT[lo:hi].T.reshape(2, 128, hi - lo)
        gidxq = np.zeros((128, IDX_TOT), dtype=np.int16)
        tglob = 0
        for k in range(1, K + 1):
            nk, npc, tk = N_K[k - 1], NPC[k - 1], TILES_K[k - 1]
            base = 512 * (k + SKIP)          # z-row offset for step k
            ridx = rand_index[k - 1, : nk * NEG].astype(np.int64) % nk
            ridx = ridx.reshape(nk, NEG)     # [i, n] source rows (pre-offset)
            for t in range(tk):
                i0 = q * npc + t * 128
                nv = max(0, min(128, npc - t * 128))
                iglob = i0 + np.arange(128)
                # pair p = n*nv + d for negs; 16*nv+d for pos
                npairs = NP_T[tglob]
                src = np.full(npairs, base, dtype=np.int16)   # pads: any valid row
                src[:16 * nv] = (ridx[iglob[:nv], :].T.reshape(16 * nv) + base
                                 ).astype(np.int16)
                src[16 * nv:17 * nv] = (iglob[:nv] + base).astype(np.int16)
                gidxq[:, IC_OFF[tglob]:IC_OFF[tglob + 1]] = np.tile(
                    src.reshape(npairs // 16, 16).T, (8, 1))
                tglob += 1
        in_maps.append({"zt": zT, "wt": wth, "ctxT": ctxTq, "gidx": gidxq})
    return in_maps


def kernel(z, c, W, rand_index):
    from concourse.bass_utils import run_bass_kernel_spmd

    if "nc" not in _CACHE:
        _CACHE["nc"] = _build()
    nc = _CACHE["nc"]
    in_maps = _host_prep(
        np.asarray(z, dtype=np.float32),
        np.asarray(c, dtype=np.float32),
        np.asarray(W, dtype=np.float32),
        np.asarray(rand_index),
    )
    res = run_bass_kernel_spmd(nc, in_maps, core_ids=list(range(NCORES)))
    _CACHE["last_res"] = res
    total = 0.0
    for r in res.results:
        total += r["out"].astype(np.float64).sum()
    return np.float32(total)
